# revision 5
# baseline (speedup 1.0000x reference)
"""Causal self-attention on 8 TRN2 cores — v2 (S^T main pass, no P transposes).

Per core (batch b, head group g of 4 heads):
  A.  qk^T = Wqk.T @ x^T (dim-major, f32r; q pre-scaled by 8)
      v    = x @ Wv (token-major bf16, with a ones column per head: [64h|1])
  S1. stats: q-major S chunks (f32r) -> per-row -max (fp16, no mask needed)
      flip -max vectors into per-head bias rows via one strided DMA
  S2. main: S^T slabs [128k, 1024q] = K^T.T Q^T + ones x biasrow (rank-1,
      fp16, tile_position row 64) -> PSUM holds x - max directly
      exp (ACT, PSUM->SBUF bf16) -> P^T tiles; zero invalid blocks;
      triangular mask on diagonal blocks (DVE mul)
  PV. y^T[65, 512] += V'[k,65].T P^T (ones col gives l = sum exp per q)
  PR. per-head out_h^T = Wp_h.T y_h^T (unnormalized)
Host: out = sum_h out_h^T.T / l_h + b_proj (+ v-bias correction).
"""
import numpy as np

import concourse.bacc as bacc
import concourse.mybir as mybir
from concourse import tile
from concourse.bass_utils import run_bass_kernel_spmd

F32 = mybir.dt.float32
F32R = mybir.dt.float32r
BF16 = mybir.dt.bfloat16
FP16 = mybir.dt.float16
AF = mybir.ActivationFunctionType
ALU = mybir.AluOpType
AX = mybir.AxisListType

T = 2048
C = 1024
HG = 4
NCT = C // 128   # 8
NQT = T // 128   # 16


def build_attention(reps: int = 1):
    nc = bacc.Bacc("TRN2", target_bir_lowering=False, debug=False)
    xt_d = nc.dram_tensor("xt", (C, T), F32R, kind="ExternalInput")
    wqk_d = nc.dram_tensor("wqk", (C, 512), F32R, kind="ExternalInput")
    wv_d = nc.dram_tensor("wv", (C, 256), F32R, kind="ExternalInput")
    bqk_d = nc.dram_tensor("bqk", (128, 4), F32, kind="ExternalInput")
    wp_d = nc.dram_tensor("wp", (HG, 64, 1024), F32R, kind="ExternalInput")
    m01_d = nc.dram_tensor("m01t", (128, 128), F32, kind="ExternalInput")
    mask_d = nc.dram_tensor("masks", (4, 128, 512), F32, kind="ExternalInput")
    out_d = nc.dram_tensor("outh", (HG, C, T), F32, kind="ExternalOutput")
    l_d = nc.dram_tensor("lsum", (HG, T), F32, kind="ExternalOutput")

    with tile.TileContext(nc) as tc:
        with (
            tc.tile_pool(name="wpool", bufs=1) as wpool,
            tc.tile_pool(name="qkv", bufs=1) as qkv,
            tc.tile_pool(name="work", bufs=1) as work,
            tc.tile_pool(name="ps", bufs=1, space="PSUM") as ps,
        ):
            # ---- static weights/consts ----
            wqk_t = [wpool.tile([128, 512], F32R, name=f"wqk{k}", tag="wqk", bufs=NCT)
                     for k in range(NCT)]
            wv_t = [wpool.tile([128, 256], F32R, name=f"wv{k}", tag="wv", bufs=NCT)
                    for k in range(NCT)]
            wph_t = [wpool.tile([64, 1024], F32R, name=f"wph{k}", tag="wph", bufs=4)
                     for k in range(HG)]
            bqk_t = wpool.tile([128, 4], F32)
            m01f = wpool.tile([128, 128], F32)
            ones16 = wpool.tile([128, 128], FP16)
            id16 = wpool.tile([128, 128], FP16)
            for k in range(NCT):
                nc.gpsimd.dma_start(wqk_t[k][:], wqk_d[k * 128:(k + 1) * 128, :])
                nc.gpsimd.dma_start(wv_t[k][:], wv_d[k * 128:(k + 1) * 128, :])
            for k in range(HG):
                nc.gpsimd.dma_start(wph_t[k][:], wp_d[k])
            nc.gpsimd.dma_start(bqk_t[:], bqk_d[:, :])
            nc.gpsimd.dma_start(m01f[:], m01_d[:, :])
            mask_t = [wpool.tile([128, 512], F32, name=f"mask{r}", tag="mask", bufs=4)
                      for r in range(4)]
            for r in range(4):
                nc.gpsimd.dma_start(mask_t[r][:], mask_d[r])
            nc.vector.memset(ones16[:], 1.0)
            from concourse import masks as _masks
            _masks.make_identity(nc, id16[:])

            def body():
                # ---- A: qkv, streaming xt by 512-token chunks ----
                qkT = [qkv.tile([128, T], F32R, name=f"qkT{m}", tag="qkT", bufs=4)
                       for m in range(4)]
                v_t = [qkv.tile([128, 260], BF16, name=f"v{t}", tag="v", bufs=NQT)
                       for t in range(NQT)]
                for t in range(NQT):
                    nc.vector.memset(v_t[t][:, :], 1.0)  # ones cols preset
                for n in range(4):
                    xt_n = [work.tile([128, 512], F32R, name=f"xt{k}_{n}",
                                      tag="xt", bufs=8)
                            for k in range(NCT)]
                    for k in range(NCT):
                        nc.gpsimd.dma_start(
                            xt_n[k][:], xt_d[k * 128:(k + 1) * 128,
                                             n * 512:(n + 1) * 512])
                    for m in range(4):
                        pmm = ps.tile([128, 512], F32, name=f"pqk{m}_{n}",
                                      tag="mm", bufs=2)
                        for k in range(NCT):
                            nc.tensor.matmul(
                                pmm[:], wqk_t[k][:, m * 128:(m + 1) * 128],
                                xt_n[k][:], start=(k == 0), stop=(k == NCT - 1),
                            )
                        nc.scalar.activation(
                            qkT[m][:, n * 512:(n + 1) * 512], pmm[:],
                            AF.Identity, bias=bqk_t[:, m:m + 1], scale=1.0,
                        )
                    for ti in range(4):
                        t = 4 * n + ti
                        pmm = ps.tile([128, 256], F32, name=f"pv{t}",
                                      tag="mm", bufs=2)
                        for k in range(NCT):
                            nc.tensor.matmul(
                                pmm[:], xt_n[k][:, ti * 128:(ti + 1) * 128],
                                wv_t[k][:], start=(k == 0), stop=(k == NCT - 1),
                            )
                        # [128, 4, 64] -> strided [128, 4(x65), 64] head cols
                        nc.scalar.copy(
                            v_t[t][:].rearrange("p (h e) -> p h e", h=4, e=65)[
                                :, :, 0:64],
                            pmm[:].rearrange("p (h e) -> p h e", h=4, e=64),
                        )

                # ---- S1 + S2 + PV per head ----
                for h in range(HG):
                    mq = h // 2
                    rq = (h % 2) * 64
                    qsl_all = qkT[mq][rq:rq + 64, :]
                    ksl_all = qkT[2 + mq][rq:rq + 64, :]

                    # S1: stats (row maxes, negated, fp16)
                    bp = 64 if rq == 0 else 0  # bias-row partition
                    qbias = work.tile([128, T], FP16, name=f"qb{h}",
                                      tag="qb", bufs=2)
                    nmqs = []
                    for qt in range(NQT):
                        wrow = (qt // 4 + 1) * 512
                        nslab = (wrow + 1023) // 1024
                        srow = [
                            ps.tile([128, 1024], F32, name=f"s{h}_{qt}_{sl}",
                                    tag="st", bufs=3)
                            for sl in range(nslab)
                        ]
                        for j in range(wrow // 512):
                            nc.tensor.matmul(
                                srow[j // 2][:, (j % 2) * 512:(j % 2 + 1) * 512],
                                qsl_all[:, qt * 128:(qt + 1) * 128],
                                ksl_all[:, j * 512:(j + 1) * 512],
                                start=True, stop=True,
                            )
                        jd = wrow // 512 - 1  # diag chunk: mask garbage above diag
                        dsl = srow[jd // 2][:, (jd % 2) * 512:(jd % 2 + 1) * 512]
                        nc.vector.tensor_tensor(
                            out=dsl, in0=dsl, in1=mask_t[qt % 4][:], op=ALU.add
                        )
                        nmq = work.tile([128, bp + 1], FP16, name=f"nm{h}_{qt}",
                                        tag="nmq", bufs=18)
                        nmqs.append(nmq)
                        if bp:
                            nc.vector.memset(nmq[:, 0:bp], 0.0)
                        if nslab > 1:
                            cm = work.tile([128, 2], F32, name=f"cm{h}_{qt}",
                                           tag="cm", bufs=3)
                            for sl in range(nslab):
                                w = min(1024, wrow - sl * 1024)
                                nc.vector.reduce_max(
                                    out=cm[:, sl:sl + 1], in_=srow[sl][:, :w],
                                    axis=AX.X,
                                )
                            nc.vector.tensor_reduce(
                                out=nmq[:, bp:bp + 1], in_=cm[:, :nslab],
                                op=ALU.max, axis=AX.X, negate=True,
                            )
                        else:
                            nc.vector.tensor_reduce(
                                out=nmq[:, bp:bp + 1], in_=srow[0][:, :wrow],
                                op=ALU.max, axis=AX.X, negate=True,
                            )
                    # flips after all stats: PE never stalls on a fresh max
                    for qt in range(NQT):
                        tb = ps.tile([bp + 1, 128], FP16, name=f"tb{h}_{qt}",
                                     tag="mm", bufs=2)
                        nc.tensor.transpose(tb[:], nmqs[qt][:], id16[:])
                        nc.vector.tensor_copy(
                            qbias[bp:bp + 1, qt * 128:(qt + 1) * 128],
                            tb[bp:bp + 1, :],
                        )

                    # S2 + PV, slab by slab
                    pt_rows = {}
                    for s in range(2):
                        for kt in range(8 * s + 8):
                            if (kt * 128) // 1024 > s:
                                continue  # entire slab below diagonal impossible
                            # causal: row kt needs q >= kt*128; slab covers
                            # q in [s*1024, s*1024+1024)
                            if kt * 128 >= (s + 1) * 1024:
                                continue
                            pt = work.tile([128, 1024], BF16,
                                           name=f"pt{h}_{s}_{kt}",
                                           tag="pt", bufs=18)
                            pt_rows[(s, kt)] = pt
                            stp = ps.tile([128, 1024], F32, name=f"stp{h}_{s}_{kt}",
                                          tag="st", bufs=3)
                            # which 512-chunks of this slab are (partly) causal
                            for jc in range(2):
                                q0 = s * 1024 + jc * 512
                                if q0 + 512 <= kt * 128:
                                    continue  # fully below diagonal
                                nc.tensor.matmul(
                                    stp[:, jc * 512:(jc + 1) * 512],
                                    ksl_all[:, kt * 128:(kt + 1) * 128],
                                    qsl_all[:, q0:q0 + 512],
                                    start=True, stop=False,
                                )
                                nc.tensor.matmul(
                                    stp[:, jc * 512:(jc + 1) * 512],
                                    ones16[bp:bp + 1, 0:128],
                                    qbias[bp:bp + 1, q0:q0 + 512],
                                    start=False, stop=True,
                                    tile_position=(bp, 0),
                                )
                            jlo = 0 if kt * 128 < s * 1024 + 512 else 1
                            qb0 = s * 8  # first q-block in slab
                            dqi = kt - qb0  # diag block index in slab
                            if 0 <= dqi < 8:
                                # additive causal mask on the diagonal block
                                nc.vector.tensor_tensor(
                                    out=stp[:, dqi * 128:(dqi + 1) * 128],
                                    in0=stp[:, dqi * 128:(dqi + 1) * 128],
                                    in1=m01f[:], op=ALU.add,
                                )
                            nc.scalar.activation(
                                pt[:, jlo * 512:1024],
                                stp[:, jlo * 512:1024],
                                AF.Exp, bias=0.0, scale=1.0,
                            )
                            # zero fully-invalid 128-blocks (k > q; exp there
                            # may be Inf garbage - overwritten, not multiplied)
                            if jlo * 4 < dqi:
                                nc.vector.memset(
                                    pt[:, jlo * 512:dqi * 128], 0.0)
                        for cc in range(2):
                            c = 2 * s + cc
                            yps = ps.tile([65, 512], F32, name=f"y{h}_{c}",
                                          tag="mm", bufs=2)
                            nkt = 4 * (c + 1)
                            for kt in range(nkt):
                                nc.tensor.matmul(
                                    yps[:],
                                    v_t[kt][:, 65 * h:65 * h + 65],
                                    pt_rows[(s, kt)][:, cc * 512:(cc + 1) * 512],
                                    start=(kt == 0), stop=(kt == nkt - 1),
                                )
                            y4 = qkv.tile([128, 512], F32R, name=f"y4_{h}_{c}",
                                          tag="y4", bufs=6)
                            nc.scalar.copy(y4[0:65, :], yps[:])
                            for m in range(8):
                                pmo = ps.tile([128, 512], F32, name=f"po{h}_{c}_{m}",
                                              tag="mm", bufs=2)
                                nc.tensor.matmul(
                                    pmo[:],
                                    wph_t[h][:, m * 128:(m + 1) * 128],
                                    y4[0:64, :], start=True, stop=True,
                                )
                                ot = work.tile([128, 512], F32, name=f"ot{h}{c}{m}",
                                               tag="ot", bufs=3)
                                nc.scalar.copy(ot[:], pmo[:])
                                nc.gpsimd.dma_start(
                                    out_d[h, m * 128:(m + 1) * 128,
                                          c * 512:(c + 1) * 512],
                                    ot[:],
                                )
                            nc.gpsimd.dma_start(
                                l_d[h, c * 512:(c + 1) * 512], y4[64:65, :]
                            )

            if reps == 1:
                body()
            else:
                with tc.For_i(0, reps):
                    body()
    nc.compile()
    return nc


_NC_CACHE = {}


def _get_nc(reps=1):
    if reps not in _NC_CACHE:
        _NC_CACHE[reps] = build_attention(reps)
    return _NC_CACHE[reps]


def _prep_inputs(x, W_attn, b_attn, W_proj):
    in_maps = []
    cc = np.arange(128)[None, :]
    pp = np.arange(128)[:, None]
    m01t = np.where(pp <= cc, 0.0, -1.0e30).astype(np.float32)  # [k,q] additive
    for core in range(8):
        b, g = core // 4, core % 4
        xt = np.ascontiguousarray(x[b].T)
        wq = W_attn[:, 256 * g:256 * g + 256] * 8.0
        wk = W_attn[:, 1024 + 256 * g:1024 + 256 * g + 256]
        wqk = np.ascontiguousarray(np.concatenate([wq, wk], axis=1))
        wv = np.ascontiguousarray(W_attn[:, 2048 + 256 * g:2048 + 256 * g + 256])
        bq = b_attn[256 * g:256 * g + 256] * 8.0
        bk = b_attn[1024 + 256 * g:1024 + 256 * g + 256]
        bqk = np.concatenate([bq, bk]).reshape(4, 128).T
        wp = W_proj[256 * g:256 * g + 256, :]
        wpp = np.ascontiguousarray(wp.reshape(HG, 64, 1024))
        masks = np.zeros((4, 128, 512), dtype=np.float32)
        c5 = np.arange(512)[None, :]
        p5 = np.arange(128)[:, None]
        for r in range(4):
            masks[r] = np.where(c5 <= p5 + 128 * r, 0.0, -1.0e30)
        in_maps.append({
            "xt": xt.astype(np.float32),
            "wqk": wqk.astype(np.float32),
            "wv": wv.astype(np.float32),
            "bqk": np.ascontiguousarray(bqk).astype(np.float32),
            "wp": wpp.astype(np.float32),
            "m01t": m01t,
            "masks": masks,
        })
    return in_maps


def kernel(x, W_attn, b_attn, W_proj, b_proj):
    x = np.asarray(x, dtype=np.float32)
    W_attn = np.asarray(W_attn, dtype=np.float32)
    b_attn = np.asarray(b_attn, dtype=np.float32)
    W_proj = np.asarray(W_proj, dtype=np.float32)
    b_proj = np.asarray(b_proj, dtype=np.float32)

    nc = _get_nc(1)
    in_maps = _prep_inputs(x, W_attn, b_attn, W_proj)
    res = run_bass_kernel_spmd(nc, in_maps, core_ids=list(range(8)))

    bv = b_attn[2048:]
    corr = bv @ W_proj
    out = np.empty((2, T, C), dtype=np.float32)
    for b in range(2):
        acc = np.zeros((C, T), dtype=np.float64)
        for g in range(4):
            r = res.results[b * 4 + g]
            for h in range(HG):
                acc += r["outh"][h] / r["lsum"][h][None, :]
        out[b] = acc.T.astype(np.float32) + b_proj + corr
    return out
